# revision 30
# baseline (speedup 1.0000x reference)
"""Trainium2 Bass kernel for nn_Attention_72404558676364.

Math: the reference computes
    pre[l,b,:] = hs_encoder[l,b,:] @ We.T + (hidden @ Wh.T + b_att)[b,:]
    attn[b,l]  = pre[l,b,:] . v
    out        = softmax(attn, axis=l)
Softmax over l is shift-invariant, so the hidden/Wh/b_att term (constant in
l for fixed b) cancels exactly and the einsum collapses to a single matvec:
    attn[b,l] = hs_encoder[l,b,:] . w_eff,   w_eff = We.T @ v
The device does one pass over hs_encoder plus the small We.T @ v, then a
per-batch softmax.  All arithmetic is fp32.

Sharding: data-parallel over batch; core c handles batches [8c, 8c+8).
hs_encoder shards are pre-transposed on the host to [H, Bc*L] so every DMA is
contiguous per partition (fp32 cannot use the DMA-transpose xbar and
strided-AP transposes are ~19x slower).

PE: fp32 matmul runs at 1/4 rate and M=1 uses one array column, so four
independent M=1 matmuls are packed into the four 32-column groups of the PE
array via tile_position — the four batches of a group accumulate concurrently
into rows 0/32/64/96 of one PSUM bank.
"""

import sys

import numpy as np

for _p in (
    "/root/.axon_site",
    "/root/.axon_site/_ro/trn_rl_repo",
    "/root/.axon_site/_ro/pypackages",
):
    if _p not in sys.path:
        sys.path.append(_p)

import concourse.bass as bass
import concourse.mybir as mybir
import concourse.tile as tile
from concourse.bass_utils import run_bass_kernel_spmd

H = 1024
L = 512
B = 64
NCORES = 8
BC = B // NCORES  # batches per core
P = 128
HC = H // P  # 128-wide chunks of the contraction dim

F32 = mybir.dt.float32

_split_n = 0


def _split_multi_waits(nc):
    """Hoist extra sem waits onto same-engine NOPs.

    The walrus build in this container rejects any instruction carrying more
    than one sync-wait ("Too many sync wait commands"), but Tile emits
    multi-wait instructions whenever one op depends on several producers.
    A NOP on the same engine immediately before the instruction waits
    equivalently (per-engine program order).
    """
    global _split_n
    engines = [
        mybir.EngineType.SP,
        mybir.EngineType.Activation,
        mybir.EngineType.DVE,
        mybir.EngineType.PE,
        mybir.EngineType.Pool,
    ]
    for fn in nc.m.functions:
        for blk in fn.blocks:
            new_insts = []
            for inst in blk.instructions:
                si = getattr(inst, "sync_info", None)
                if si is not None and si.on_wait and len(si.on_wait) > 1:
                    waits = list(si.on_wait)
                    si.on_wait = waits[:1]
                    # The exit drain carries one wait per DMA queue sem; its
                    # waits may run on ANY engine because the all-engine
                    # barrier right after it orders everything.  Mid-kernel
                    # instructions need same-engine NOPs (program order).
                    wide = (
                        isinstance(inst, mybir.InstDrain) and len(waits) > 3
                    )
                    for k, w in enumerate(waits[1:]):
                        _split_n += 1
                        eng = engines[k % len(engines)] if wide else inst.engine
                        new_insts.append(
                            mybir.InstNoOp(
                                name=f"I-wsplit-{_split_n}",
                                engine=eng,
                                sync_info=mybir.SyncInfo(
                                    on_wait=[w], on_update=[]
                                ),
                                bass_nofuse=True,
                            )
                        )
                new_insts.append(inst)
            blk.instructions = new_insts


def _build():
    nc = bass.Bass(target_bir_lowering=False, enable_partition_id=False)
    hsT = nc.dram_tensor("hsT", [H, BC * L], F32, kind="ExternalInput")
    we = nc.dram_tensor("We", [H, H], F32, kind="ExternalInput")
    v = nc.dram_tensor("v", [P, HC], F32, kind="ExternalInput")
    out = nc.dram_tensor("out", [BC, L], F32, kind="ExternalOutput")

    with tile.TileContext(nc) as tc:
        with (
            tc.tile_pool(name="singles", bufs=1) as singles,
            tc.tile_pool(name="hs", bufs=8) as hs_pool,
            tc.tile_pool(name="srow", bufs=3) as srow_pool,
            tc.tile_pool(name="psw", bufs=1, space="PSUM") as psw_pool,
            tc.tile_pool(name="pst", bufs=1, space="PSUM") as pst_pool,
            tc.tile_pool(name="pss", bufs=2, space="PSUM") as pss_pool,
            tc.tile_pool(name="psq", bufs=4, space="PSUM") as psq_pool,
        ):
            # ---- small operands ---------------------------------------
            v_sb = singles.tile([P, HC], F32)
            nc.sync.dma_start(out=v_sb[:], in_=v[:])
            ident = singles.tile([1, 1], F32)
            nc.vector.memset(ident[:], 1.0)

            # Per-chunk We DMAs (first flip matmul starts after ~1 us),
            # alternating between the two HWDGE rings.
            we_sb = singles.tile([P, HC, H], F32)
            for hc in range(HC):
                eng = nc.sync if hc % 2 == 0 else nc.scalar
                eng.dma_start(
                    out=we_sb[:, hc, :], in_=we[hc * P : (hc + 1) * P, :]
                )

            # ---- w_eff = We.T @ v as a [1, H] fp32 row ----------------
            # lhsT = v chunk [128,1]; rhs = We chunk [128, 512]; the two
            # k-halves run concurrently on PE column-groups 0 and 1,
            # accumulating into rows 0 and 32 of one PSUM bank.
            w_row = singles.tile([1, H], F32)
            ph = psw_pool.tile([P, L], F32)
            for hc in range(HC):
                for half in range(2):
                    nc.tensor.matmul(
                        ph[32 * half : 32 * half + 1, :],
                        lhsT=v_sb[:, hc : hc + 1],
                        rhs=we_sb[:, hc, half * L : (half + 1) * L],
                        start=(hc == 0),
                        stop=(hc == HC - 1),
                        tile_position=(0, 32 * half),
                    )
            for half in range(2):
                nc.scalar.copy(
                    out=w_row[0:1, half * L : (half + 1) * L],
                    in_=ph[32 * half : 32 * half + 1, :],
                )

            # ---- w_row -> w_cols[p, hc] = w_eff[hc*128+p] -------------
            w_cols = singles.tile([P, HC], F32)
            for hc in range(HC):
                pt = pst_pool.tile([P, 1], F32)
                nc.tensor.transpose(
                    pt[:], w_row[0:1, hc * P : (hc + 1) * P], ident[:]
                )
                nc.vector.tensor_copy(out=w_cols[:, hc : hc + 1], in_=pt[:])

            # ---- scores[j, l] = hsT[:, j*L+l] . w_eff ------------------
            # Batch groups of (4, 3, 1): each group's batches run
            # concurrently on PE column-groups into one PSUM bank, and the
            # final single-batch group leaves only one softmax chain
            # exposed after the last matmul.
            groups = [(0, 3), (3, 4), (7, 1)]
            for gi, (j0, ng) in enumerate(groups):
                tiles = []
                for hc in range(HC):
                    eng = nc.sync if hc % 2 == 0 else nc.scalar
                    t = hs_pool.tile([P, ng * L], F32, tag=f"hs{ng}")
                    eng.dma_start(
                        out=t[:],
                        in_=hsT[
                            hc * P : (hc + 1) * P, j0 * L : (j0 + ng) * L
                        ],
                    )
                    tiles.append(t)
                if ng == 1:
                    # Single batch: split the k-contraction over the four PE
                    # column-groups (2 chunks each) so the exposed tail
                    # matmuls still run 4-way.  Each partial row gets its OWN
                    # psum bank: row q's PE writes finish after chunk 2q+1,
                    # and separate banks let the bank-level dependency
                    # tracker start its add immediately instead of after the
                    # whole group's matmuls.
                    ps_q = [
                        psq_pool.tile([P, L], F32, name=f"psq{q}", tag="psq")
                        for q in range(4)
                    ]
                    for hc in range(HC):
                        q = hc // 2
                        nc.tensor.matmul(
                            ps_q[q][32 * q : 32 * q + 1, :],
                            lhsT=w_cols[:, hc : hc + 1],
                            rhs=tiles[hc][:, 0:L],
                            start=(hc % 2 == 0),
                            stop=(hc % 2 == 1),
                            tile_position=(0, 32 * q),
                        )
                else:
                    ps = pss_pool.tile([P, L], F32)
                    # Skewed wavefront: batch g's accumulation closes g steps
                    # early, so its softmax chain overlaps the remaining
                    # batches' matmuls instead of stacking after them.
                    for step in range(HC + ng - 1):
                        for g in range(ng):
                            hc = step - g
                            if not 0 <= hc < HC:
                                continue
                            nc.tensor.matmul(
                                ps[32 * g : 32 * g + 1, :],
                                lhsT=w_cols[:, hc : hc + 1],
                                rhs=tiles[hc][:, g * L : (g + 1) * L],
                                start=(hc == 0),
                                stop=(hc == HC - 1),
                                tile_position=(0, 32 * g),
                            )
                for g in range(ng):
                    j = j0 + g
                    # Per-batch softmax on idle DVE/ACT while later batches'
                    # matmuls stream, reading scores straight from PSUM.
                    if ng == 1:
                        acc = srow_pool.tile([1, L], F32)
                        nc.scalar.copy(out=acc[:], in_=ps_q[0][0:1, :])
                        for q in range(1, 4):
                            nc.vector.tensor_add(
                                out=acc[:], in0=acc[:],
                                in1=ps_q[q][32 * q : 32 * q + 1, :],
                            )
                        row = acc[:]
                    else:
                        row = ps[32 * g : 32 * g + 1, :]
                    negmax = srow_pool.tile([1, 1], F32)
                    nc.vector.reduce_max(
                        out=negmax[:], in_=row, axis=mybir.AxisListType.X,
                        negate=True,
                    )
                    exps = srow_pool.tile([1, L], F32)
                    sums = srow_pool.tile([1, 1], F32)
                    nc.scalar.activation(
                        out=exps[:],
                        in_=row,
                        func=mybir.ActivationFunctionType.Exp,
                        bias=negmax[:],
                        scale=1.0,
                        accum_out=sums[:],
                    )
                    rsum = srow_pool.tile([1, 1], F32)
                    nc.vector.reciprocal(out=rsum[:], in_=sums[:])
                    orow = srow_pool.tile([1, L], F32)
                    nc.vector.tensor_scalar_mul(
                        out=orow[:], in0=exps[:], scalar1=rsum[:]
                    )
                    if gi == len(groups) - 1:
                        # rings are idle at the tail; HWDGE has the lower
                        # first-byte latency
                        nc.sync.dma_start(out=out[j : j + 1, :], in_=orow[:])
                    else:
                        # SWDGE keeps mid-stream stores off the HWDGE rings
                        # so their waits never stall the input DMAs.
                        nc.gpsimd.dma_start(out=out[j : j + 1, :], in_=orow[:])

    _split_multi_waits(nc)
    return nc


_NC_CACHE = None


def _make_in_maps(hs_encoder, W_att, vector):
    hs_encoder = np.ascontiguousarray(hs_encoder, dtype=np.float32)
    we_np = np.ascontiguousarray(W_att[:, H:], dtype=np.float32)
    v_np = np.ascontiguousarray(
        np.asarray(vector, dtype=np.float32)[:, 0].reshape(HC, P).T
    )

    in_maps = []
    for c in range(NCORES):
        shard = hs_encoder[:, c * BC : (c + 1) * BC, :]  # [L, BC, H]
        hst = np.ascontiguousarray(shard.transpose(2, 1, 0).reshape(H, BC * L))
        in_maps.append({"hsT": hst, "We": we_np, "v": v_np})
    return in_maps


def kernel(hidden, hs_encoder, W_att, b_att, vector):
    global _NC_CACHE
    if _NC_CACHE is None:
        _NC_CACHE = _build()
    nc = _NC_CACHE

    in_maps = _make_in_maps(hs_encoder, W_att, vector)
    res = run_bass_kernel_spmd(nc, in_maps, core_ids=list(range(NCORES)))
    out = np.concatenate([res.results[c]["out"] for c in range(NCORES)], axis=0)
    return out[:, None, :].astype(np.float32)


# revision 31
# speedup vs baseline: 1.0196x; 1.0196x over previous
"""Trainium2 Bass kernel for nn_Attention_72404558676364.

Math: the reference computes
    pre[l,b,:] = hs_encoder[l,b,:] @ We.T + (hidden @ Wh.T + b_att)[b,:]
    attn[b,l]  = pre[l,b,:] . v
    out        = softmax(attn, axis=l)
Softmax over l is shift-invariant, so the hidden/Wh/b_att term (constant in
l for fixed b) cancels exactly and the einsum collapses to a single matvec:
    attn[b,l] = hs_encoder[l,b,:] . w_eff,   w_eff = We.T @ v
The device does one pass over hs_encoder plus the small We.T @ v, then a
per-batch softmax.  All arithmetic is fp32.

Sharding: data-parallel over batch; core c handles batches [8c, 8c+8).
hs_encoder shards are pre-transposed on the host to [H, Bc*L] so every DMA is
contiguous per partition (fp32 cannot use the DMA-transpose xbar and
strided-AP transposes are ~19x slower).

PE: fp32 matmul runs at 1/4 rate and M=1 uses one array column, so four
independent M=1 matmuls are packed into the four 32-column groups of the PE
array via tile_position — the four batches of a group accumulate concurrently
into rows 0/32/64/96 of one PSUM bank.
"""

import sys

import numpy as np

for _p in (
    "/root/.axon_site",
    "/root/.axon_site/_ro/trn_rl_repo",
    "/root/.axon_site/_ro/pypackages",
):
    if _p not in sys.path:
        sys.path.append(_p)

import concourse.bass as bass
import concourse.mybir as mybir
import concourse.tile as tile
from concourse.bass_utils import run_bass_kernel_spmd

H = 1024
L = 512
B = 64
NCORES = 8
BC = B // NCORES  # batches per core
P = 128
HC = H // P  # 128-wide chunks of the contraction dim

F32 = mybir.dt.float32

_split_n = 0


def _split_multi_waits(nc):
    """Hoist extra sem waits onto same-engine NOPs.

    The walrus build in this container rejects any instruction carrying more
    than one sync-wait ("Too many sync wait commands"), but Tile emits
    multi-wait instructions whenever one op depends on several producers.
    A NOP on the same engine immediately before the instruction waits
    equivalently (per-engine program order).
    """
    global _split_n
    engines = [
        mybir.EngineType.SP,
        mybir.EngineType.Activation,
        mybir.EngineType.DVE,
        mybir.EngineType.PE,
        mybir.EngineType.Pool,
    ]
    for fn in nc.m.functions:
        for blk in fn.blocks:
            new_insts = []
            for inst in blk.instructions:
                si = getattr(inst, "sync_info", None)
                if si is not None and si.on_wait and len(si.on_wait) > 1:
                    waits = list(si.on_wait)
                    si.on_wait = waits[:1]
                    # The exit drain carries one wait per DMA queue sem; its
                    # waits may run on ANY engine because the all-engine
                    # barrier right after it orders everything.  Mid-kernel
                    # instructions need same-engine NOPs (program order).
                    wide = (
                        isinstance(inst, mybir.InstDrain) and len(waits) > 3
                    )
                    for k, w in enumerate(waits[1:]):
                        _split_n += 1
                        eng = engines[k % len(engines)] if wide else inst.engine
                        new_insts.append(
                            mybir.InstNoOp(
                                name=f"I-wsplit-{_split_n}",
                                engine=eng,
                                sync_info=mybir.SyncInfo(
                                    on_wait=[w], on_update=[]
                                ),
                                bass_nofuse=True,
                            )
                        )
                new_insts.append(inst)
            blk.instructions = new_insts


def _build():
    nc = bass.Bass(target_bir_lowering=False, enable_partition_id=False)
    hsT = nc.dram_tensor("hsT", [H, BC * L], F32, kind="ExternalInput")
    we = nc.dram_tensor("We", [H, H], F32, kind="ExternalInput")
    v = nc.dram_tensor("v", [P, HC], F32, kind="ExternalInput")
    out = nc.dram_tensor("out", [BC, L], F32, kind="ExternalOutput")

    with tile.TileContext(nc) as tc:
        with (
            tc.tile_pool(name="singles", bufs=1) as singles,
            tc.tile_pool(name="hs", bufs=8) as hs_pool,
            tc.tile_pool(name="srow", bufs=5) as srow_pool,
            tc.tile_pool(name="psw", bufs=1, space="PSUM") as psw_pool,
            tc.tile_pool(name="pst", bufs=1, space="PSUM") as pst_pool,
            tc.tile_pool(name="pss", bufs=2, space="PSUM") as pss_pool,
            tc.tile_pool(name="psq", bufs=4, space="PSUM") as psq_pool,
        ):
            # ---- small operands ---------------------------------------
            v_sb = singles.tile([P, HC], F32)
            nc.sync.dma_start(out=v_sb[:], in_=v[:])
            ident = singles.tile([1, 1], F32)
            nc.vector.memset(ident[:], 1.0)

            # Per-chunk We DMAs (first flip matmul starts after ~1 us),
            # alternating between the two HWDGE rings.
            we_sb = singles.tile([P, HC, H], F32)
            for hc in range(HC):
                eng = nc.sync if hc % 2 == 0 else nc.scalar
                eng.dma_start(
                    out=we_sb[:, hc, :], in_=we[hc * P : (hc + 1) * P, :]
                )

            # ---- w_eff = We.T @ v as a [1, H] fp32 row ----------------
            # lhsT = v chunk [128,1]; rhs = We chunk [128, 512]; the two
            # k-halves run concurrently on PE column-groups 0 and 1,
            # accumulating into rows 0 and 32 of one PSUM bank.
            w_row = singles.tile([1, H], F32)
            ph = psw_pool.tile([P, L], F32)
            for hc in range(HC):
                for half in range(2):
                    nc.tensor.matmul(
                        ph[32 * half : 32 * half + 1, :],
                        lhsT=v_sb[:, hc : hc + 1],
                        rhs=we_sb[:, hc, half * L : (half + 1) * L],
                        start=(hc == 0),
                        stop=(hc == HC - 1),
                        tile_position=(0, 32 * half),
                    )
            for half in range(2):
                nc.scalar.copy(
                    out=w_row[0:1, half * L : (half + 1) * L],
                    in_=ph[32 * half : 32 * half + 1, :],
                )

            # ---- w_row -> w_cols[p, hc] = w_eff[hc*128+p] -------------
            w_cols = singles.tile([P, HC], F32)
            for hc in range(HC):
                pt = pst_pool.tile([P, 1], F32)
                nc.tensor.transpose(
                    pt[:], w_row[0:1, hc * P : (hc + 1) * P], ident[:]
                )
                nc.vector.tensor_copy(out=w_cols[:, hc : hc + 1], in_=pt[:])

            # ---- scores[j, l] = hsT[:, j*L+l] . w_eff ------------------
            # Batch groups of (4, 3, 1): each group's batches run
            # concurrently on PE column-groups into one PSUM bank, and the
            # final single-batch group leaves only one softmax chain
            # exposed after the last matmul.
            groups = [(0, 3), (3, 4), (7, 1)]
            for gi, (j0, ng) in enumerate(groups):
                tiles = []
                for hc in range(HC):
                    eng = nc.sync if hc % 2 == 0 else nc.scalar
                    t = hs_pool.tile([P, ng * L], F32, tag=f"hs{ng}")
                    eng.dma_start(
                        out=t[:],
                        in_=hsT[
                            hc * P : (hc + 1) * P, j0 * L : (j0 + ng) * L
                        ],
                    )
                    tiles.append(t)
                if ng == 1:
                    # Single batch: split the k-contraction over the four PE
                    # column-groups (2 chunks each) so the exposed tail
                    # matmuls still run 4-way.  Each partial row gets its OWN
                    # psum bank: row q's PE writes finish after chunk 2q+1,
                    # and separate banks let the bank-level dependency
                    # tracker start its add immediately instead of after the
                    # whole group's matmuls.
                    ps_q = [
                        psq_pool.tile([P, L], F32, name=f"psq{q}", tag="psq")
                        for q in range(4)
                    ]
                    for hc in range(HC):
                        q = hc // 2
                        nc.tensor.matmul(
                            ps_q[q][32 * q : 32 * q + 1, :],
                            lhsT=w_cols[:, hc : hc + 1],
                            rhs=tiles[hc][:, 0:L],
                            start=(hc % 2 == 0),
                            stop=(hc % 2 == 1),
                            tile_position=(0, 32 * q),
                        )
                else:
                    ps = pss_pool.tile([P, L], F32)
                    # Skewed wavefront: batch g's accumulation closes g steps
                    # early, so its softmax chain overlaps the remaining
                    # batches' matmuls instead of stacking after them.
                    for step in range(HC + ng - 1):
                        for g in range(ng):
                            hc = step - g
                            if not 0 <= hc < HC:
                                continue
                            nc.tensor.matmul(
                                ps[32 * g : 32 * g + 1, :],
                                lhsT=w_cols[:, hc : hc + 1],
                                rhs=tiles[hc][:, g * L : (g + 1) * L],
                                start=(hc == 0),
                                stop=(hc == HC - 1),
                                tile_position=(0, 32 * g),
                            )
                for g in range(ng):
                    j = j0 + g
                    # Per-batch softmax on idle DVE/ACT while later batches'
                    # matmuls stream, reading scores straight from PSUM.
                    if ng == 1:
                        acc = srow_pool.tile([1, L], F32)
                        nc.scalar.copy(out=acc[:], in_=ps_q[0][0:1, :])
                        for q in range(1, 4):
                            nc.vector.tensor_add(
                                out=acc[:], in0=acc[:],
                                in1=ps_q[q][32 * q : 32 * q + 1, :],
                            )
                        row = acc[:]
                    else:
                        row = ps[32 * g : 32 * g + 1, :]
                    negmax = srow_pool.tile([1, 1], F32)
                    nc.vector.reduce_max(
                        out=negmax[:], in_=row, axis=mybir.AxisListType.X,
                        negate=True,
                    )
                    exps = srow_pool.tile([1, L], F32)
                    sums = srow_pool.tile([1, 1], F32)
                    nc.scalar.activation(
                        out=exps[:],
                        in_=row,
                        func=mybir.ActivationFunctionType.Exp,
                        bias=negmax[:],
                        scale=1.0,
                        accum_out=sums[:],
                    )
                    rsum = srow_pool.tile([1, 1], F32)
                    nc.vector.reciprocal(out=rsum[:], in_=sums[:])
                    orow = srow_pool.tile([1, L], F32)
                    nc.vector.tensor_scalar_mul(
                        out=orow[:], in0=exps[:], scalar1=rsum[:]
                    )
                    if gi == len(groups) - 1:
                        # rings are idle at the tail; HWDGE has the lower
                        # first-byte latency
                        nc.sync.dma_start(out=out[j : j + 1, :], in_=orow[:])
                    else:
                        # SWDGE keeps mid-stream stores off the HWDGE rings
                        # so their waits never stall the input DMAs.
                        nc.gpsimd.dma_start(out=out[j : j + 1, :], in_=orow[:])

    _split_multi_waits(nc)
    return nc


_NC_CACHE = None


def _make_in_maps(hs_encoder, W_att, vector):
    hs_encoder = np.ascontiguousarray(hs_encoder, dtype=np.float32)
    we_np = np.ascontiguousarray(W_att[:, H:], dtype=np.float32)
    v_np = np.ascontiguousarray(
        np.asarray(vector, dtype=np.float32)[:, 0].reshape(HC, P).T
    )

    in_maps = []
    for c in range(NCORES):
        shard = hs_encoder[:, c * BC : (c + 1) * BC, :]  # [L, BC, H]
        hst = np.ascontiguousarray(shard.transpose(2, 1, 0).reshape(H, BC * L))
        in_maps.append({"hsT": hst, "We": we_np, "v": v_np})
    return in_maps


def kernel(hidden, hs_encoder, W_att, b_att, vector):
    global _NC_CACHE
    if _NC_CACHE is None:
        _NC_CACHE = _build()
    nc = _NC_CACHE

    in_maps = _make_in_maps(hs_encoder, W_att, vector)
    res = run_bass_kernel_spmd(nc, in_maps, core_ids=list(range(NCORES)))
    out = np.concatenate([res.results[c]["out"] for c in range(NCORES)], axis=0)
    return out[:, None, :].astype(np.float32)
